# revision 14
# baseline (speedup 1.0000x reference)
"""RGCN GuidanceClassifier on 8 Trainium2 NeuronCores (bf16 edition).

Node slices (and their incoming edges) partitioned across 8 cores; per
256-node window the sorted edge stream is cut into fully-packed 128-edge
chunks (chunks may span relation boundaries). Per chunk: one indirect-DMA
gather of x[src] (bf16 rows); per (chunk, relation)-segment one fused DVE
op builds sel[e,n] = (iota==dst_local)*w in bf16 (w = 1/cnt folds the
mean), then bf16 PE matmuls with 256-wide moving dims:
    aggT[din,n] += msgs.T @ sel ;  outT[dout,n] += W_r.T @ aggT
Root transform: layer 1 rides in the gather stream as self-edges (rel 8);
layers 2-3 use a persistent SBUF-resident x^T of the previous layer's
activations (written by the relu) as the moving operand of a single
W_root matmul per window. Bias = rank-1 matmul against e0. Relu writes
x^T (bf16); PE-transpose + copy produce node-major rows for the bf16
AllGather between layers; mean-pool accumulated in PSUM during layer 3,
AllReduce, then both MLP heads computed redundantly per core.
"""

import math
import os

import ml_dtypes
import numpy as np

N = 100000
E = 600000
D = 128
R = 8
B = 64
V = 5000
L = 3
NCORES = 8
S = N // NCORES          # 12500 nodes per core
WIN = 256                # nodes per window (sel moving dim)
NWIN = math.ceil(S / WIN)          # 49
NHALF = math.ceil(S / 128)         # 98 half-windows
CHUNK = 128

# AllGather pieces (in half-window units): each piece is AllGathered as soon
# as its rows are produced, overlapping the exchange with the window loop.
PIECE_HW = [0, 13, 26, 39, 52, 65, 78, 91, 98]
NPIECE = len(PIECE_HW) - 1
PIECE_ROWS = [min(PIECE_HW[p + 1] * 128, S) - PIECE_HW[p] * 128
              for p in range(NPIECE)]
PIECE_BASE = [8 * int(b) for b in
              np.cumsum([0] + PIECE_ROWS[:-1], dtype=np.int64)]

LAST_RESULTS = None


def _xg_lut():
    """Global node id -> row in the piece-major exchanged-x layout
    xg[[piece][core][row_in_piece]]."""
    pos = np.arange(S)
    hw = pos // 128
    p = np.searchsorted(np.asarray(PIECE_HW[1:]), hw, side="right")
    rows_p = np.asarray(PIECE_ROWS)[p]
    base = np.asarray(PIECE_BASE)[p]
    off = pos - 128 * np.asarray(PIECE_HW)[p]
    lut = np.empty(N, np.int64)
    for c in range(NCORES):
        lut[c * S:(c + 1) * S] = base + c * rows_p + off
    return lut


def _streams(node_type, edge_index, edge_type, batch):
    """Per-core per-window sorted edge streams; L1 stream appends the
    self-edges (rel=8), the L2/3 stream does not (root via SBUF x^T)."""
    src = edge_index[0].astype(np.int64)
    dst = edge_index[1].astype(np.int64)
    rel = edge_type.astype(np.int64)

    cnt = np.zeros((N, R), np.float32)
    np.add.at(cnt, (dst, rel), 1.0)
    w_edge = (1.0 / np.maximum(cnt, 1.0))[dst, rel].astype(np.float32)
    nt = node_type.astype(np.int64)

    core = dst // S
    dloc = dst - core * S
    win = dloc // WIN

    lut = _xg_lut()
    stA = [[None] * NWIN for _ in range(NCORES)]   # L1 (edges + self)
    stB = [[None] * NWIN for _ in range(NCORES)]   # L2/3 (edges only)
    for c in range(NCORES):
        m = core == c
        s_c, d_c, r_c, w_c, wi_c = src[m], dloc[m], rel[m], w_edge[m], win[m]
        order = np.lexsort((d_c, r_c, wi_c))
        s_c, d_c, r_c, w_c, wi_c = (a[order] for a in (s_c, d_c, r_c, w_c, wi_c))
        bounds = np.searchsorted(wi_c, np.arange(NWIN + 1))
        for w in range(NWIN):
            lo, hi = bounds[w], bounds[w + 1]
            dl = d_c[lo:hi] - w * WIN
            nwn = min(WIN, S - w * WIN)
            gids = c * S + w * WIN + np.arange(nwn)
            stA[c][w] = (
                np.concatenate([nt[s_c[lo:hi]], nt[gids]]),
                np.concatenate([dl, np.arange(nwn)]).astype(np.float32),
                np.concatenate([w_c[lo:hi], np.ones(nwn, np.float32)]),
                np.concatenate([r_c[lo:hi], np.full(nwn, R)]),
            )
            stB[c][w] = (lut[s_c[lo:hi]], dl.astype(np.float32), w_c[lo:hi],
                         r_c[lo:hi])
    return stA, stB, cnt


def _grid(streams):
    """Shared chunk/segment structure (union over cores)."""
    nch = np.zeros(NWIN, np.int64)
    for w in range(NWIN):
        mx = max(len(streams[c][w][0]) for c in range(NCORES))
        nch[w] = max(1, math.ceil(mx / CHUNK))
    segs = [[] for _ in range(NWIN)]
    for w in range(NWIN):
        present = set()
        for c in range(NCORES):
            r_arr = streams[c][w][3]
            for k in range(int(nch[w])):
                rr = np.unique(r_arr[k * CHUNK:(k + 1) * CHUNK])
                for r in rr:
                    present.add((k, int(r)))
        segs[w] = sorted(present)
    chunk_cols = []
    seg_cols = []
    for w in range(NWIN):
        for k in range(int(nch[w])):
            chunk_cols.append((w, k))
        for (k, r) in segs[w]:
            seg_cols.append((w, k, r))
    return nch, segs, chunk_cols, seg_cols


def _fill(streams, nch, chunk_cols, seg_cols):
    CC, CS = len(chunk_cols), len(seg_cols)
    srci = np.zeros((NCORES, 128, CC), np.int32)
    dstf = np.zeros((NCORES, 128, CS), np.float32)
    wv = np.zeros((NCORES, 128, CS), np.float32)
    for c in range(NCORES):
        for j, (w, k) in enumerate(chunk_cols):
            s_arr = streams[c][w][0]
            seg = s_arr[k * CHUNK:(k + 1) * CHUNK]
            srci[c, :len(seg), j] = seg
        for j, (w, k, r) in enumerate(seg_cols):
            s_arr, d_arr, w_arr, r_arr = streams[c][w]
            sl = slice(k * CHUNK, (k + 1) * CHUNK)
            d_s, w_s, r_s = d_arr[sl], w_arr[sl], r_arr[sl]
            m = r_s == r
            kk = len(d_s)
            dstf[c, :kk, j] = np.where(m, d_s, 0.0)
            wv[c, :kk, j] = np.where(m, w_s, 0.0)
    return srci, dstf, wv


def _preprocess(node_type, edge_index, edge_type, batch):
    stA, stB, _ = _streams(node_type, edge_index, edge_type, batch)
    gA = _grid(stA)
    gB = _grid(stB)
    mA = _fill(stA, gA[0], gA[2], gA[3])
    mB = _fill(stB, gB[0], gB[2], gB[3])

    bcnt = np.zeros(B, np.float64)
    np.add.at(bcnt, batch.astype(np.int64), 1.0)
    inv_b = (1.0 / np.maximum(bcnt, 1.0)).astype(np.float32)
    batchf = np.full((NCORES, 128, NHALF), -1.0, np.float32)
    invcb = np.zeros((NCORES, 128, NHALF), np.float32)
    for c in range(NCORES):
        ids = batch[c * S:(c + 1) * S].astype(np.int64)
        for j in range(NHALF):
            seg = ids[j * 128:(j + 1) * 128]
            k = len(seg)
            batchf[c, :k, j] = seg.astype(np.float32)
            invcb[c, :k, j] = inv_b[seg]
    return gA, gB, mA, mB, batchf, invcb


def _build_program(gA, gB, CCA, CSA, CCB, CSB):
    import concourse.bass as bass
    import concourse.bacc as bacc
    import concourse.mybir as mybir
    import concourse.tile as tile
    from concourse.masks import make_identity

    f32 = mybir.dt.float32
    bf16 = mybir.dt.bfloat16
    fp8 = mybir.dt.float8e4
    i32 = mybir.dt.int32
    AF = mybir.ActivationFunctionType
    OP = mybir.AluOpType

    XT_COLS = NWIN * WIN       # 12544 (padded S)

    nc = bacc.Bacc("TRN2", target_bir_lowering=False, debug=False,
                   num_devices=NCORES)

    t_emb = nc.dram_tensor("node_emb", [V, D], bf16, kind="ExternalInput")
    t_wpack = nc.dram_tensor("wpack", [L, 128, 10 * 128], bf16,
                             kind="ExternalInput")
    t_srcA = nc.dram_tensor("srcA", [128, CCA], i32, kind="ExternalInput")
    t_dstfA = nc.dram_tensor("dstfA", [128, CSA], f32, kind="ExternalInput")
    t_wvA = nc.dram_tensor("wvA", [128, CSA], f32, kind="ExternalInput")
    t_srcB = nc.dram_tensor("srcB", [128, CCB], i32, kind="ExternalInput")
    t_dstfB = nc.dram_tensor("dstfB", [128, CSB], f32, kind="ExternalInput")
    t_wvB = nc.dram_tensor("wvB", [128, CSB], f32, kind="ExternalInput")
    t_batchf = nc.dram_tensor("batchf", [128, NHALF], f32, kind="ExternalInput")
    t_invcb = nc.dram_tensor("invcb", [128, NHALF], f32, kind="ExternalInput")
    t_iota = nc.dram_tensor("iota", [128, WIN], bf16, kind="ExternalInput")
    t_e0 = nc.dram_tensor("e0", [128, WIN], bf16, kind="ExternalInput")
    t_rw1 = nc.dram_tensor("rw1", [128, 128], f32, kind="ExternalInput")
    t_sw1 = nc.dram_tensor("sw1", [128, 128], f32, kind="ExternalInput")
    t_w2p = nc.dram_tensor("w2p", [128, 2], f32, kind="ExternalInput")
    t_b1p = nc.dram_tensor("b1p", [128, 2], f32, kind="ExternalInput")
    t_b2p = nc.dram_tensor("b2p", [64, 2], f32, kind="ExternalInput")
    t_out = nc.dram_tensor("out", [64, 2], f32, kind="ExternalOutput")

    with tile.TileContext(nc) as tc:
        with tc.tile_pool(name="static", bufs=1) as st, \
             tc.tile_pool(name="wt", bufs=2) as wtp, \
             tc.tile_pool(name="msgs", bufs=14) as msgsp, \
             tc.tile_pool(name="sel", bufs=8) as selp, \
             tc.tile_pool(name="aggsb", bufs=4) as aggsbp, \
             tc.tile_pool(name="xo", bufs=4) as xop, \
             tc.tile_pool(name="pagg", bufs=3, space="PSUM") as paggp, \
             tc.tile_pool(name="pout", bufs=2, space="PSUM") as poutp, \
             tc.tile_pool(name="ptr", bufs=2, space="PSUM") as ptrp, \
             tc.tile_pool(name="pg", bufs=1, space="PSUM") as pgp, \
             tc.tile_pool(name="dram", bufs=1, space="DRAM") as dram:

            srcA_t = st.tile([128, CCA], i32)
            dstfA_t = st.tile([128, CSA], f32)
            wvA_t = st.tile([128, CSA], f32)
            srcB_t = st.tile([128, CCB], i32)
            dstfB_t = st.tile([128, CSB], f32)
            wvB_t = st.tile([128, CSB], f32)
            batchf_t = st.tile([128, NHALF], f32)
            invcb_t = st.tile([128, NHALF], f32)
            iota_t = st.tile([128, WIN], bf16)
            e0_t = st.tile([128, WIN], bf16)
            ident_t = st.tile([128, 128], bf16)
            for dst_t, src_t in ((srcA_t, t_srcA), (dstfA_t, t_dstfA),
                                 (wvA_t, t_wvA), (srcB_t, t_srcB),
                                 (dstfB_t, t_dstfB), (wvB_t, t_wvB),
                                 (batchf_t, t_batchf), (invcb_t, t_invcb),
                                 (iota_t, t_iota), (e0_t, t_e0)):
                nc.sync.dma_start(dst_t[:], src_t[:])
            make_identity(nc, ident_t[:])

            # persistent x^T of previous / current layer (bf16)
            xT = [st.tile([128, XT_COLS], bf16, name=f"xT{i}") for i in range(2)]

            ag_in = [dram.tile([S, D], fp8, tag=f"agin{l}", name=f"agin{l}")
                     for l in range(2)]
            # one Shared tile per AllGather piece (single-writer rule), plus
            # a plain DRAM concat that the next layer's gathers index into
            ag_pc = [[dram.tile([8 * PIECE_ROWS[p], D], fp8,
                                addr_space="Shared", tag=f"agp{l}_{p}",
                                name=f"agp{l}_{p}") for p in range(NPIECE)]
                     for l in range(2)]
            ag_out = [dram.tile([N, D], fp8, tag=f"agout{l}",
                                name=f"agout{l}") for l in range(2)]
            pg = pgp.tile([128, B], f32)
            piece_ends = {PIECE_HW[p + 1]: p for p in range(NPIECE)}

            for l in range(L):
                wtile = wtp.tile([128, 10 * 128], bf16)
                nc.sync.dma_start(wtile[:], t_wpack[l])

                if l == 0:
                    nchs, segss, chunk_cols, seg_cols = gA
                    src_t, dstf_t, wv_t = srcA_t, dstfA_t, wvA_t
                    xsrc = t_emb
                else:
                    nchs, segss, chunk_cols, seg_cols = gB
                    src_t, dstf_t, wv_t = srcB_t, dstfB_t, wvB_t
                    xsrc = ag_out[l - 1]
                ch_col = {wk: j for j, wk in enumerate(chunk_cols)}
                sg_col = {wkr: j for j, wkr in enumerate(seg_cols)}
                xT_cur = xT[l % 2]
                xT_prev = xT[(l + 1) % 2]

                for w in range(NWIN):
                    poutT = poutp.tile([128, WIN], f32)
                    nc.tensor.matmul(
                        poutT[:], lhsT=wtile[:, 9 * 128:10 * 128],
                        rhs=e0_t[:], start=True, stop=False)
                    if l > 0:
                        # root transform from SBUF-resident prev x^T
                        nc.tensor.matmul(
                            poutT[:], lhsT=wtile[:, R * 128:(R + 1) * 128],
                            rhs=xT_prev[:, w * WIN:(w + 1) * WIN],
                            start=False, stop=False)

                    # gather all chunks of this window
                    msgs_tiles = []
                    for k in range(int(nchs[w])):
                        msgs = msgsp.tile([128, 128], bf16 if l == 0 else fp8,
                                          name=f"msgs{l}_{w}_{k}", tag="msgs")
                        nc.gpsimd.indirect_dma_start(
                            out=msgs[:], out_offset=None, in_=xsrc[:],
                            in_offset=bass.IndirectOffsetOnAxis(
                                ap=src_t[:, ch_col[(w, k)]:ch_col[(w, k)] + 1],
                                axis=0))
                        msgs_tiles.append(msgs)

                    by_rel = {}
                    for (k, r) in segss[w]:
                        by_rel.setdefault(r, []).append(k)
                    rlist = sorted(by_rel.keys())
                    nrel = len(rlist)
                    for ri, r in enumerate(rlist):
                        ks = by_rel[r]
                        paggT = paggp.tile([128, WIN], f32, tag="paggT",
                                           name=f"paggT{l}_{w}_{r}")
                        for i, k in enumerate(ks):
                            j = sg_col[(w, k, r)]
                            sel = selp.tile([128, WIN], bf16,
                                            name=f"sel{l}_{w}_{r}_{i}",
                                            tag="sel")
                            nc.vector.tensor_scalar(
                                out=sel[:], in0=iota_t[:],
                                scalar1=dstf_t[:, j:j + 1],
                                scalar2=wv_t[:, j:j + 1],
                                op0=OP.is_equal, op1=OP.mult)
                            nc.tensor.matmul(
                                paggT[:], lhsT=msgs_tiles[k][:], rhs=sel[:],
                                start=(i == 0), stop=(i == len(ks) - 1))
                        aggsb = aggsbp.tile([128, WIN], bf16, tag="aggsb",
                                            name=f"aggsb{l}_{w}_{r}")
                        if r % 2 == 0:
                            nc.scalar.activation(aggsb[:], paggT[:], AF.Copy)
                        else:
                            nc.vector.tensor_copy(aggsb[:], paggT[:])
                        nc.tensor.matmul(
                            poutT[:], lhsT=wtile[:, r * 128:(r + 1) * 128],
                            rhs=aggsb[:], start=False,
                            stop=(ri == nrel - 1))

                    # relu -> persistent x^T slice (bf16)
                    nc.scalar.activation(
                        xT_cur[:, w * WIN:(w + 1) * WIN], poutT[:], AF.Relu)

                    nh = min(2, math.ceil((S - w * WIN) / 128))
                    for h in range(nh):
                        rows = min(128, S - (w * WIN + h * 128))
                        ptr = ptrp.tile([128, 128], bf16, tag="ptr",
                                        name=f"ptr{l}_{w}_{h}")
                        nc.tensor.transpose(
                            ptr[:],
                            xT_cur[:, w * WIN + h * 128:w * WIN + (h + 1) * 128],
                            ident_t[:])
                        xo = xop.tile([128, 128], fp8 if l < 2 else bf16,
                                      tag="xo", name=f"xo{l}_{w}_{h}")
                        nc.vector.tensor_copy(xo[:], ptr[:])
                        if l < 2:
                            hw_ = w * 2 + h
                            nc.sync.dma_start(
                                ag_in[l][w * WIN + h * 128:
                                         w * WIN + h * 128 + rows, :],
                                xo[:rows, :])
                            if hw_ + 1 in piece_ends:
                                p = piece_ends[hw_ + 1]
                                a = 128 * PIECE_HW[p]
                                nc.gpsimd.collective_compute(
                                    "AllGather", mybir.AluOpType.bypass,
                                    replica_groups=[list(range(NCORES))],
                                    ins=[ag_in[l][a:a + PIECE_ROWS[p], :]],
                                    outs=[ag_pc[l][p][:]])
                                nc.scalar.dma_start(
                                    ag_out[l][PIECE_BASE[p]:
                                              PIECE_BASE[p] + 8 * PIECE_ROWS[p],
                                              :],
                                    ag_pc[l][p][:])
                        else:
                            hw_ = w * 2 + h
                            selb = selp.tile([128, B], bf16, tag="selb",
                                             name=f"selb{w}_{h}")
                            nc.vector.tensor_scalar(
                                out=selb[:], in0=iota_t[:, :B],
                                scalar1=batchf_t[:, hw_:hw_ + 1],
                                scalar2=invcb_t[:, hw_:hw_ + 1],
                                op0=OP.is_equal, op1=OP.mult)
                            nc.tensor.matmul(
                                pg[:], lhsT=xo[:], rhs=selb[:],
                                start=(hw_ == 0), stop=(hw_ == NHALF - 1))

            # heads
            rw1_t = st.tile([128, 128], f32)
            sw1_t = st.tile([128, 128], f32)
            w2p_t = st.tile([128, 2], f32)
            b1p_t = st.tile([128, 2], f32)
            b2p_t = st.tile([64, 2], f32)
            nc.sync.dma_start(rw1_t[:], t_rw1[:])
            nc.sync.dma_start(sw1_t[:], t_sw1[:])
            nc.sync.dma_start(w2p_t[:], t_w2p[:])
            nc.sync.dma_start(b1p_t[:], t_b1p[:])
            nc.sync.dma_start(b2p_t[:], t_b2p[:])

            pgsb = st.tile([128, B], f32)
            nc.vector.tensor_copy(pgsb[:], pg[:])
            ar_in = dram.tile([128, B], f32, tag="arin")
            ar_out = dram.tile([128, B], f32, addr_space="Shared", tag="arout")
            nc.sync.dma_start(ar_in[:], pgsb[:])
            nc.gpsimd.collective_compute(
                "AllReduce", mybir.AluOpType.add,
                replica_groups=[list(range(NCORES))],
                ins=[ar_in[:]], outs=[ar_out[:]])
            gT = st.tile([128, B], f32)
            nc.sync.dma_start(gT[:], ar_out[:])

            ph2 = ptrp.tile([64, 2], f32, tag="ptr")
            for ci, w1t in enumerate((rw1_t, sw1_t)):
                ph = paggp.tile([128, B], f32, tag="paggT",
                                name=f"ph{ci}")
                nc.tensor.matmul(ph[:], lhsT=w1t[:], rhs=gT[:],
                                 start=True, stop=True)
                hT = st.tile([128, B], f32, tag=f"hT{ci}", name=f"hT{ci}")
                nc.scalar.activation(hT[:], ph[:], AF.Relu,
                                     bias=b1p_t[:, ci:ci + 1])
                nc.tensor.matmul(ph2[:, ci:ci + 1], lhsT=hT[:],
                                 rhs=w2p_t[:, ci:ci + 1],
                                 start=True, stop=True)
            outsb = st.tile([64, 2], f32)
            nc.vector.tensor_add(outsb[:], ph2[:], b2p_t[:])
            nc.sync.dma_start(t_out[:], outsb[:])

    nc.compile()
    return nc


def kernel(node_type, edge_index, edge_type, batch, node_emb, rel_w, root_w,
           bias, risk_w1, risk_b1, risk_w2, risk_b2, safe_w1, safe_b1,
           safe_w2, safe_b2):
    global LAST_RESULTS
    import concourse.bass_utils as bass_utils

    node_type = np.asarray(node_type, np.int32)
    edge_index = np.asarray(edge_index, np.int32)
    edge_type = np.asarray(edge_type, np.int32)
    batch = np.asarray(batch, np.int32)
    node_emb = np.asarray(node_emb, np.float32)
    rel_w = np.asarray(rel_w, np.float32)
    root_w = np.asarray(root_w, np.float32)
    bias_np = np.asarray(bias, np.float32)

    gA, gB, mA, mB, batchf, invcb = _preprocess(
        node_type, edge_index, edge_type, batch)
    srcA, dstfA, wvA = mA
    srcB, dstfB, wvB = mB

    nc = _build_program(gA, gB, srcA.shape[2], dstfA.shape[2],
                        srcB.shape[2], dstfB.shape[2])

    bf = ml_dtypes.bfloat16
    wpack = np.zeros((L, 10, 128, 128), np.float32)
    wpack[:, :R] = rel_w
    wpack[:, R] = root_w
    wpack[:, 9, 0, :] = bias_np
    wpack = np.ascontiguousarray(wpack.transpose(0, 2, 1, 3)).reshape(
        L, 128, 10 * 128).astype(bf)

    iota = np.tile(np.arange(WIN, dtype=np.float32), (128, 1)).astype(bf)
    e0 = np.zeros((128, WIN), np.float32)
    e0[0, :] = 1.0
    e0 = e0.astype(bf)
    w2p = np.stack([np.asarray(risk_w2, np.float32)[:, 0],
                    np.asarray(safe_w2, np.float32)[:, 0]], axis=1)
    b1p = np.stack([np.asarray(risk_b1, np.float32),
                    np.asarray(safe_b1, np.float32)], axis=1)
    b2p = np.stack([np.full(64, np.float32(np.asarray(risk_b2)[0])),
                    np.full(64, np.float32(np.asarray(safe_b2)[0]))], axis=1)

    shared = dict(node_emb=node_emb.astype(bf), wpack=wpack, iota=iota, e0=e0,
                  rw1=np.asarray(risk_w1, np.float32),
                  sw1=np.asarray(safe_w1, np.float32),
                  w2p=w2p, b1p=b1p, b2p=b2p)
    in_maps = []
    for c in range(NCORES):
        m = dict(shared)
        m.update(srcA=srcA[c], dstfA=dstfA[c], wvA=wvA[c],
                 srcB=srcB[c], dstfB=dstfB[c], wvB=wvB[c],
                 batchf=batchf[c], invcb=invcb[c])
        in_maps.append(m)

    trace = os.environ.get("KERNEL_TRACE", "0") == "1"
    res = bass_utils.run_bass_kernel_spmd(
        nc, in_maps, core_ids=list(range(NCORES)), trace=trace)
    LAST_RESULTS = res
    out = res.results[0]["out"]
    return out[:, 0].copy(), out[:, 1].copy()


# revision 18
# speedup vs baseline: 1.2051x; 1.2051x over previous
"""RGCN GuidanceClassifier on 8 Trainium2 NeuronCores (bf16 edition).

Node slices (and their incoming edges) partitioned across 8 cores; per
256-node window the sorted edge stream is cut into fully-packed 128-edge
chunks (chunks may span relation boundaries). Per chunk: one indirect-DMA
gather of x[src] (bf16 rows); per (chunk, relation)-segment one fused DVE
op builds sel[e,n] = (iota==dst_local)*w in bf16 (w = 1/cnt folds the
mean), then bf16 PE matmuls with 256-wide moving dims:
    aggT[din,n] += msgs.T @ sel ;  outT[dout,n] += W_r.T @ aggT
Root transform: layer 1 rides in the gather stream as self-edges (rel 8);
layers 2-3 use a persistent SBUF-resident x^T of the previous layer's
activations (written by the relu) as the moving operand of a single
W_root matmul per window. Bias = rank-1 matmul against e0. Relu writes
x^T (bf16); PE-transpose + copy produce node-major rows for the bf16
AllGather between layers; mean-pool accumulated in PSUM during layer 3,
AllReduce, then both MLP heads computed redundantly per core.
"""

import math
import os

import ml_dtypes
import numpy as np

N = 100000
E = 600000
D = 128
R = 8
B = 64
V = 5000
L = 3
NCORES = 8
S = N // NCORES          # 12500 nodes per core
WIN = 256                # nodes per window (sel moving dim)
NWIN = math.ceil(S / WIN)          # 49
NHALF = math.ceil(S / 128)         # 98 half-windows
CHUNK = 128

# AllGather pieces (in half-window units): each piece is AllGathered as soon
# as its rows are produced, overlapping the exchange with the window loop.
PIECE_HW = [0, 13, 26, 39, 52, 65, 78, 91, 98]
NPIECE = len(PIECE_HW) - 1
PIECE_ROWS = [min(PIECE_HW[p + 1] * 128, S) - PIECE_HW[p] * 128
              for p in range(NPIECE)]
PIECE_BASE = [8 * int(b) for b in
              np.cumsum([0] + PIECE_ROWS[:-1], dtype=np.int64)]

LAST_RESULTS = None


def _xg_lut():
    """Global node id -> row in the piece-major exchanged-x layout
    xg[[piece][core][row_in_piece]]."""
    pos = np.arange(S)
    hw = pos // 128
    p = np.searchsorted(np.asarray(PIECE_HW[1:]), hw, side="right")
    rows_p = np.asarray(PIECE_ROWS)[p]
    base = np.asarray(PIECE_BASE)[p]
    off = pos - 128 * np.asarray(PIECE_HW)[p]
    lut = np.empty(N, np.int64)
    for c in range(NCORES):
        lut[c * S:(c + 1) * S] = base + c * rows_p + off
    return lut


def _streams(node_type, edge_index, edge_type, batch):
    """Per-core per-window sorted edge streams; L1 stream appends the
    self-edges (rel=8), the L2/3 stream does not (root via SBUF x^T)."""
    src = edge_index[0].astype(np.int64)
    dst = edge_index[1].astype(np.int64)
    rel = edge_type.astype(np.int64)

    cnt = np.zeros((N, R), np.float32)
    np.add.at(cnt, (dst, rel), 1.0)
    w_edge = (1.0 / np.maximum(cnt, 1.0))[dst, rel].astype(np.float32)
    nt = node_type.astype(np.int64)

    core = dst // S
    dloc = dst - core * S
    win = dloc // WIN

    lut = _xg_lut()
    stA = [[None] * NWIN for _ in range(NCORES)]   # L1 (edges + self)
    stB = [[None] * NWIN for _ in range(NCORES)]   # L2/3 (edges only)
    for c in range(NCORES):
        m = core == c
        s_c, d_c, r_c, w_c, wi_c = src[m], dloc[m], rel[m], w_edge[m], win[m]
        order = np.lexsort((d_c, r_c, wi_c))
        s_c, d_c, r_c, w_c, wi_c = (a[order] for a in (s_c, d_c, r_c, w_c, wi_c))
        bounds = np.searchsorted(wi_c, np.arange(NWIN + 1))
        for w in range(NWIN):
            lo, hi = bounds[w], bounds[w + 1]
            dl = d_c[lo:hi] - w * WIN
            nwn = min(WIN, S - w * WIN)
            gids = c * S + w * WIN + np.arange(nwn)
            stA[c][w] = (
                np.concatenate([nt[s_c[lo:hi]], nt[gids]]),
                np.concatenate([dl, np.arange(nwn)]).astype(np.float32),
                np.concatenate([w_c[lo:hi], np.ones(nwn, np.float32)]),
                np.concatenate([r_c[lo:hi], np.full(nwn, R)]),
            )
            stB[c][w] = (lut[s_c[lo:hi]], dl.astype(np.float32), w_c[lo:hi],
                         r_c[lo:hi])
    return stA, stB, cnt


def _grid(streams):
    """Shared chunk/segment structure (union over cores)."""
    nch = np.zeros(NWIN, np.int64)
    for w in range(NWIN):
        mx = max(len(streams[c][w][0]) for c in range(NCORES))
        nch[w] = max(1, math.ceil(mx / CHUNK))
    segs = [[] for _ in range(NWIN)]
    for w in range(NWIN):
        present = set()
        for c in range(NCORES):
            r_arr = streams[c][w][3]
            for k in range(int(nch[w])):
                rr = np.unique(r_arr[k * CHUNK:(k + 1) * CHUNK])
                for r in rr:
                    present.add((k, int(r)))
        segs[w] = sorted(present)
    chunk_cols = []
    seg_cols = []
    for w in range(NWIN):
        for k in range(int(nch[w])):
            chunk_cols.append((w, k))
        for (k, r) in segs[w]:
            seg_cols.append((w, k, r))
    return nch, segs, chunk_cols, seg_cols


def _fill(streams, nch, chunk_cols, seg_cols):
    CC, CS = len(chunk_cols), len(seg_cols)
    srci = np.zeros((NCORES, 128, CC), np.int32)
    dstf = np.zeros((NCORES, 128, CS), np.float32)
    wv = np.zeros((NCORES, 128, CS), np.float32)
    for c in range(NCORES):
        for j, (w, k) in enumerate(chunk_cols):
            s_arr = streams[c][w][0]
            seg = s_arr[k * CHUNK:(k + 1) * CHUNK]
            srci[c, :len(seg), j] = seg
        for j, (w, k, r) in enumerate(seg_cols):
            s_arr, d_arr, w_arr, r_arr = streams[c][w]
            sl = slice(k * CHUNK, (k + 1) * CHUNK)
            d_s, w_s, r_s = d_arr[sl], w_arr[sl], r_arr[sl]
            m = r_s == r
            kk = len(d_s)
            dstf[c, :kk, j] = np.where(m, d_s, 0.0)
            wv[c, :kk, j] = np.where(m, w_s, 0.0)
    return srci, dstf, wv


def _preprocess(node_type, edge_index, edge_type, batch):
    stA, stB, _ = _streams(node_type, edge_index, edge_type, batch)
    gA = _grid(stA)
    gB = _grid(stB)
    mA = _fill(stA, gA[0], gA[2], gA[3])
    mB = _fill(stB, gB[0], gB[2], gB[3])

    bcnt = np.zeros(B, np.float64)
    np.add.at(bcnt, batch.astype(np.int64), 1.0)
    inv_b = (1.0 / np.maximum(bcnt, 1.0)).astype(np.float32)
    batchf = np.full((NCORES, 128, NHALF), -1.0, np.float32)
    invcb = np.zeros((NCORES, 128, NHALF), np.float32)
    for c in range(NCORES):
        ids = batch[c * S:(c + 1) * S].astype(np.int64)
        for j in range(NHALF):
            seg = ids[j * 128:(j + 1) * 128]
            k = len(seg)
            batchf[c, :k, j] = seg.astype(np.float32)
            invcb[c, :k, j] = inv_b[seg]
    return gA, gB, mA, mB, batchf, invcb


def _build_program(gA, gB, CCA, CSA, CCB, CSB):
    import concourse.bass as bass
    import concourse.bacc as bacc
    import concourse.mybir as mybir
    import concourse.tile as tile
    from concourse.masks import make_identity

    f32 = mybir.dt.float32
    bf16 = mybir.dt.bfloat16
    fp8 = mybir.dt.float8e4
    i32 = mybir.dt.int32
    AF = mybir.ActivationFunctionType
    OP = mybir.AluOpType

    XT_COLS = NWIN * WIN       # 12544 (padded S)

    nc = bacc.Bacc("TRN2", target_bir_lowering=False, debug=False,
                   num_devices=NCORES)

    t_emb = nc.dram_tensor("node_emb", [V, D], bf16, kind="ExternalInput")
    t_wpack = nc.dram_tensor("wpack", [L, 128, 10 * 128], bf16,
                             kind="ExternalInput")
    t_srcA = nc.dram_tensor("srcA", [128, CCA], i32, kind="ExternalInput")
    t_dstfA = nc.dram_tensor("dstfA", [128, CSA], f32, kind="ExternalInput")
    t_wvA = nc.dram_tensor("wvA", [128, CSA], f32, kind="ExternalInput")
    t_srcB = nc.dram_tensor("srcB", [128, CCB], i32, kind="ExternalInput")
    t_dstfB = nc.dram_tensor("dstfB", [128, CSB], f32, kind="ExternalInput")
    t_wvB = nc.dram_tensor("wvB", [128, CSB], f32, kind="ExternalInput")
    t_batchf = nc.dram_tensor("batchf", [128, NHALF], f32, kind="ExternalInput")
    t_invcb = nc.dram_tensor("invcb", [128, NHALF], f32, kind="ExternalInput")
    t_iota = nc.dram_tensor("iota", [128, WIN], bf16, kind="ExternalInput")
    t_e0 = nc.dram_tensor("e0", [128, WIN], bf16, kind="ExternalInput")
    t_rw1 = nc.dram_tensor("rw1", [128, 128], f32, kind="ExternalInput")
    t_sw1 = nc.dram_tensor("sw1", [128, 128], f32, kind="ExternalInput")
    t_w2p = nc.dram_tensor("w2p", [128, 2], f32, kind="ExternalInput")
    t_b1p = nc.dram_tensor("b1p", [128, 2], f32, kind="ExternalInput")
    t_b2p = nc.dram_tensor("b2p", [64, 2], f32, kind="ExternalInput")
    t_out = nc.dram_tensor("out", [64, 2], f32, kind="ExternalOutput")

    with tile.TileContext(nc) as tc:
        with tc.tile_pool(name="static", bufs=1) as st, \
             tc.tile_pool(name="wt", bufs=2) as wtp, \
             tc.tile_pool(name="msgs", bufs=14) as msgsp, \
             tc.tile_pool(name="sel", bufs=8) as selp, \
             tc.tile_pool(name="aggsb", bufs=4) as aggsbp, \
             tc.tile_pool(name="xo", bufs=4) as xop, \
             tc.tile_pool(name="pagg", bufs=3, space="PSUM") as paggp, \
             tc.tile_pool(name="pout", bufs=2, space="PSUM") as poutp, \
             tc.tile_pool(name="ptr", bufs=2, space="PSUM") as ptrp, \
             tc.tile_pool(name="pg", bufs=1, space="PSUM") as pgp, \
             tc.tile_pool(name="dram", bufs=1, space="DRAM") as dram:

            srcA_t = st.tile([128, CCA], i32)
            dstfA_t = st.tile([128, CSA], f32)
            wvA_t = st.tile([128, CSA], f32)
            srcB_t = st.tile([128, CCB], i32)
            dstfB_t = st.tile([128, CSB], f32)
            wvB_t = st.tile([128, CSB], f32)
            batchf_t = st.tile([128, NHALF], f32)
            invcb_t = st.tile([128, NHALF], f32)
            iota_t = st.tile([128, WIN], bf16)
            e0_t = st.tile([128, WIN], bf16)
            ident_t = st.tile([128, 128], bf16)
            for dst_t, src_t in ((srcA_t, t_srcA), (dstfA_t, t_dstfA),
                                 (wvA_t, t_wvA), (srcB_t, t_srcB),
                                 (dstfB_t, t_dstfB), (wvB_t, t_wvB),
                                 (batchf_t, t_batchf), (invcb_t, t_invcb),
                                 (iota_t, t_iota), (e0_t, t_e0)):
                nc.sync.dma_start(dst_t[:], src_t[:])
            make_identity(nc, ident_t[:])

            # persistent x^T of previous / current layer (bf16)
            xT = [st.tile([128, XT_COLS], bf16, name=f"xT{i}") for i in range(2)]

            ag_in = [dram.tile([S, D], fp8, tag=f"agin{l}", name=f"agin{l}")
                     for l in range(2)]
            # non-Shared output: pieces AllGather directly into disjoint
            # slices (Shared space enforces a single writing instruction)
            ag_out = [dram.tile([N, D], fp8, tag=f"agout{l}",
                                name=f"agout{l}") for l in range(2)]
            pg = pgp.tile([128, B], f32)
            # fire piece p's AllGather a few half-windows after its rows are
            # produced so the issuing sequencer never stalls on the DMA sems;
            # pieces whose (delayed) fire point falls past the loop flush at
            # the layer end.
            DELAY_HW = 4
            fire_at = {}
            for p in range(NPIECE):
                t = PIECE_HW[p + 1] - 1 + DELAY_HW
                if t < NHALF - 1:
                    fire_at[t] = p

            def _ag_piece(l, p):
                a = 128 * PIECE_HW[p]
                nc.gpsimd.collective_compute(
                    "AllGather", mybir.AluOpType.bypass,
                    replica_groups=[list(range(NCORES))],
                    ins=[ag_in[l][a:a + PIECE_ROWS[p], :]],
                    outs=[ag_out[l][PIECE_BASE[p]:
                                    PIECE_BASE[p] + 8 * PIECE_ROWS[p], :]])

            for l in range(L):
                wtile = wtp.tile([128, 10 * 128], bf16)
                nc.sync.dma_start(wtile[:], t_wpack[l])

                if l == 0:
                    nchs, segss, chunk_cols, seg_cols = gA
                    src_t, dstf_t, wv_t = srcA_t, dstfA_t, wvA_t
                    xsrc = t_emb
                else:
                    nchs, segss, chunk_cols, seg_cols = gB
                    src_t, dstf_t, wv_t = srcB_t, dstfB_t, wvB_t
                    xsrc = ag_out[l - 1]
                ch_col = {wk: j for j, wk in enumerate(chunk_cols)}
                sg_col = {wkr: j for j, wkr in enumerate(seg_cols)}
                xT_cur = xT[l % 2]
                xT_prev = xT[(l + 1) % 2]

                for w in range(NWIN):
                    poutT = poutp.tile([128, WIN], f32)
                    nc.tensor.matmul(
                        poutT[:], lhsT=wtile[:, 9 * 128:10 * 128],
                        rhs=e0_t[:], start=True, stop=False)
                    if l > 0:
                        # root transform from SBUF-resident prev x^T
                        nc.tensor.matmul(
                            poutT[:], lhsT=wtile[:, R * 128:(R + 1) * 128],
                            rhs=xT_prev[:, w * WIN:(w + 1) * WIN],
                            start=False, stop=False)

                    # gather all chunks of this window
                    msgs_tiles = []
                    for k in range(int(nchs[w])):
                        msgs = msgsp.tile([128, 128], bf16 if l == 0 else fp8,
                                          name=f"msgs{l}_{w}_{k}", tag="msgs")
                        nc.gpsimd.indirect_dma_start(
                            out=msgs[:], out_offset=None, in_=xsrc[:],
                            in_offset=bass.IndirectOffsetOnAxis(
                                ap=src_t[:, ch_col[(w, k)]:ch_col[(w, k)] + 1],
                                axis=0))
                        msgs_tiles.append(msgs)

                    by_rel = {}
                    for (k, r) in segss[w]:
                        by_rel.setdefault(r, []).append(k)
                    rlist = sorted(by_rel.keys())
                    nrel = len(rlist)
                    for ri, r in enumerate(rlist):
                        ks = by_rel[r]
                        paggT = paggp.tile([128, WIN], f32, tag="paggT",
                                           name=f"paggT{l}_{w}_{r}")
                        for i, k in enumerate(ks):
                            j = sg_col[(w, k, r)]
                            sel = selp.tile([128, WIN], bf16,
                                            name=f"sel{l}_{w}_{r}_{i}",
                                            tag="sel")
                            nc.vector.tensor_scalar(
                                out=sel[:], in0=iota_t[:],
                                scalar1=dstf_t[:, j:j + 1],
                                scalar2=wv_t[:, j:j + 1],
                                op0=OP.is_equal, op1=OP.mult)
                            nc.tensor.matmul(
                                paggT[:], lhsT=msgs_tiles[k][:], rhs=sel[:],
                                start=(i == 0), stop=(i == len(ks) - 1))
                        aggsb = aggsbp.tile([128, WIN], bf16, tag="aggsb",
                                            name=f"aggsb{l}_{w}_{r}")
                        if r % 2 == 0:
                            nc.scalar.activation(aggsb[:], paggT[:], AF.Copy)
                        else:
                            nc.vector.tensor_copy(aggsb[:], paggT[:])
                        nc.tensor.matmul(
                            poutT[:], lhsT=wtile[:, r * 128:(r + 1) * 128],
                            rhs=aggsb[:], start=False,
                            stop=(ri == nrel - 1))

                    # relu -> persistent x^T slice (bf16)
                    nc.scalar.activation(
                        xT_cur[:, w * WIN:(w + 1) * WIN], poutT[:], AF.Relu)

                    nh = min(2, math.ceil((S - w * WIN) / 128))
                    for h in range(nh):
                        rows = min(128, S - (w * WIN + h * 128))
                        ptr = ptrp.tile([128, 128], bf16, tag="ptr",
                                        name=f"ptr{l}_{w}_{h}")
                        nc.tensor.transpose(
                            ptr[:],
                            xT_cur[:, w * WIN + h * 128:w * WIN + (h + 1) * 128],
                            ident_t[:])
                        xo = xop.tile([128, 128], fp8 if l < 2 else bf16,
                                      tag="xo", name=f"xo{l}_{w}_{h}")
                        nc.vector.tensor_copy(xo[:], ptr[:])
                        if l < 2:
                            hw_ = w * 2 + h
                            nc.sync.dma_start(
                                ag_in[l][w * WIN + h * 128:
                                         w * WIN + h * 128 + rows, :],
                                xo[:rows, :])
                            if hw_ in fire_at:
                                _ag_piece(l, fire_at[hw_])
                        else:
                            hw_ = w * 2 + h
                            selb = selp.tile([128, B], bf16, tag="selb",
                                             name=f"selb{w}_{h}")
                            nc.vector.tensor_scalar(
                                out=selb[:], in0=iota_t[:, :B],
                                scalar1=batchf_t[:, hw_:hw_ + 1],
                                scalar2=invcb_t[:, hw_:hw_ + 1],
                                op0=OP.is_equal, op1=OP.mult)
                            nc.tensor.matmul(
                                pg[:], lhsT=xo[:], rhs=selb[:],
                                start=(hw_ == 0), stop=(hw_ == NHALF - 1))

                if l < 2:
                    for p in range(NPIECE):
                        if PIECE_HW[p + 1] - 1 + DELAY_HW >= NHALF - 1:
                            _ag_piece(l, p)

            # heads
            rw1_t = st.tile([128, 128], f32)
            sw1_t = st.tile([128, 128], f32)
            w2p_t = st.tile([128, 2], f32)
            b1p_t = st.tile([128, 2], f32)
            b2p_t = st.tile([64, 2], f32)
            nc.sync.dma_start(rw1_t[:], t_rw1[:])
            nc.sync.dma_start(sw1_t[:], t_sw1[:])
            nc.sync.dma_start(w2p_t[:], t_w2p[:])
            nc.sync.dma_start(b1p_t[:], t_b1p[:])
            nc.sync.dma_start(b2p_t[:], t_b2p[:])

            pgsb = st.tile([128, B], f32)
            nc.vector.tensor_copy(pgsb[:], pg[:])
            ar_in = dram.tile([128, B], f32, tag="arin")
            ar_out = dram.tile([128, B], f32, addr_space="Shared", tag="arout")
            nc.sync.dma_start(ar_in[:], pgsb[:])
            nc.gpsimd.collective_compute(
                "AllReduce", mybir.AluOpType.add,
                replica_groups=[list(range(NCORES))],
                ins=[ar_in[:]], outs=[ar_out[:]])
            gT = st.tile([128, B], f32)
            nc.sync.dma_start(gT[:], ar_out[:])

            ph2 = ptrp.tile([64, 2], f32, tag="ptr")
            for ci, w1t in enumerate((rw1_t, sw1_t)):
                ph = paggp.tile([128, B], f32, tag="paggT",
                                name=f"ph{ci}")
                nc.tensor.matmul(ph[:], lhsT=w1t[:], rhs=gT[:],
                                 start=True, stop=True)
                hT = st.tile([128, B], f32, tag=f"hT{ci}", name=f"hT{ci}")
                nc.scalar.activation(hT[:], ph[:], AF.Relu,
                                     bias=b1p_t[:, ci:ci + 1])
                nc.tensor.matmul(ph2[:, ci:ci + 1], lhsT=hT[:],
                                 rhs=w2p_t[:, ci:ci + 1],
                                 start=True, stop=True)
            outsb = st.tile([64, 2], f32)
            nc.vector.tensor_add(outsb[:], ph2[:], b2p_t[:])
            nc.sync.dma_start(t_out[:], outsb[:])

    nc.compile()
    return nc


def kernel(node_type, edge_index, edge_type, batch, node_emb, rel_w, root_w,
           bias, risk_w1, risk_b1, risk_w2, risk_b2, safe_w1, safe_b1,
           safe_w2, safe_b2):
    global LAST_RESULTS
    import concourse.bass_utils as bass_utils

    node_type = np.asarray(node_type, np.int32)
    edge_index = np.asarray(edge_index, np.int32)
    edge_type = np.asarray(edge_type, np.int32)
    batch = np.asarray(batch, np.int32)
    node_emb = np.asarray(node_emb, np.float32)
    rel_w = np.asarray(rel_w, np.float32)
    root_w = np.asarray(root_w, np.float32)
    bias_np = np.asarray(bias, np.float32)

    gA, gB, mA, mB, batchf, invcb = _preprocess(
        node_type, edge_index, edge_type, batch)
    srcA, dstfA, wvA = mA
    srcB, dstfB, wvB = mB

    nc = _build_program(gA, gB, srcA.shape[2], dstfA.shape[2],
                        srcB.shape[2], dstfB.shape[2])

    bf = ml_dtypes.bfloat16
    wpack = np.zeros((L, 10, 128, 128), np.float32)
    wpack[:, :R] = rel_w
    wpack[:, R] = root_w
    wpack[:, 9, 0, :] = bias_np
    wpack = np.ascontiguousarray(wpack.transpose(0, 2, 1, 3)).reshape(
        L, 128, 10 * 128).astype(bf)

    iota = np.tile(np.arange(WIN, dtype=np.float32), (128, 1)).astype(bf)
    e0 = np.zeros((128, WIN), np.float32)
    e0[0, :] = 1.0
    e0 = e0.astype(bf)
    w2p = np.stack([np.asarray(risk_w2, np.float32)[:, 0],
                    np.asarray(safe_w2, np.float32)[:, 0]], axis=1)
    b1p = np.stack([np.asarray(risk_b1, np.float32),
                    np.asarray(safe_b1, np.float32)], axis=1)
    b2p = np.stack([np.full(64, np.float32(np.asarray(risk_b2)[0])),
                    np.full(64, np.float32(np.asarray(safe_b2)[0]))], axis=1)

    shared = dict(node_emb=node_emb.astype(bf), wpack=wpack, iota=iota, e0=e0,
                  rw1=np.asarray(risk_w1, np.float32),
                  sw1=np.asarray(safe_w1, np.float32),
                  w2p=w2p, b1p=b1p, b2p=b2p)
    in_maps = []
    for c in range(NCORES):
        m = dict(shared)
        m.update(srcA=srcA[c], dstfA=dstfA[c], wvA=wvA[c],
                 srcB=srcB[c], dstfB=dstfB[c], wvB=wvB[c],
                 batchf=batchf[c], invcb=invcb[c])
        in_maps.append(m)

    trace = os.environ.get("KERNEL_TRACE", "0") == "1"
    res = bass_utils.run_bass_kernel_spmd(
        nc, in_maps, core_ids=list(range(NCORES)), trace=trace)
    LAST_RESULTS = res
    out = res.results[0]["out"]
    return out[:, 0].copy(), out[:, 1].copy()


# revision 22
# speedup vs baseline: 1.2890x; 1.0696x over previous
"""RGCN GuidanceClassifier on 8 Trainium2 NeuronCores (bf16 edition).

Node slices (and their incoming edges) partitioned across 8 cores; per
256-node window the sorted edge stream is cut into fully-packed 128-edge
chunks (chunks may span relation boundaries). Per chunk: one indirect-DMA
gather of x[src] (bf16 rows); per (chunk, relation)-segment one fused DVE
op builds sel[e,n] = (iota==dst_local)*w in bf16 (w = 1/cnt folds the
mean), then bf16 PE matmuls with 256-wide moving dims:
    aggT[din,n] += msgs.T @ sel ;  outT[dout,n] += W_r.T @ aggT
Root transform: layer 1 rides in the gather stream as self-edges (rel 8);
layers 2-3 use a persistent SBUF-resident x^T of the previous layer's
activations (written by the relu) as the moving operand of a single
W_root matmul per window. Bias = rank-1 matmul against e0. Relu writes
x^T (bf16); PE-transpose + copy produce node-major rows for the bf16
AllGather between layers; mean-pool accumulated in PSUM during layer 3,
AllReduce, then both MLP heads computed redundantly per core.
"""

import math
import os

import ml_dtypes
import numpy as np

N = 100000
E = 600000
D = 128
R = 8
B = 64
V = 5000
L = 3
NCORES = 8
S = N // NCORES          # 12500 nodes per core
WIN = 256                # nodes per window (sel moving dim)
NWIN = math.ceil(S / WIN)          # 49
NHALF = math.ceil(S / 128)         # 98 half-windows
CHUNK = 128

# AllGather pieces (in half-window units): each piece is AllGathered as soon
# as its rows are produced, overlapping the exchange with the window loop.
PIECE_HW = [0, 13, 26, 39, 52, 65, 78, 91, 98]
NPIECE = len(PIECE_HW) - 1
PIECE_ROWS = [min(PIECE_HW[p + 1] * 128, S) - PIECE_HW[p] * 128
              for p in range(NPIECE)]
PIECE_BASE = [8 * int(b) for b in
              np.cumsum([0] + PIECE_ROWS[:-1], dtype=np.int64)]

LAST_RESULTS = None


def _xg_lut():
    """Global node id -> row in the piece-major exchanged-x layout
    xg[[piece][core][row_in_piece]]."""
    pos = np.arange(S)
    hw = pos // 128
    p = np.searchsorted(np.asarray(PIECE_HW[1:]), hw, side="right")
    rows_p = np.asarray(PIECE_ROWS)[p]
    base = np.asarray(PIECE_BASE)[p]
    off = pos - 128 * np.asarray(PIECE_HW)[p]
    lut = np.empty(N, np.int64)
    for c in range(NCORES):
        lut[c * S:(c + 1) * S] = base + c * rows_p + off
    return lut


def _streams(node_type, edge_index, edge_type, batch):
    """Per-core per-window sorted edge streams; L1 stream appends the
    self-edges (rel=8), the L2/3 stream does not (root via SBUF x^T)."""
    src = edge_index[0].astype(np.int64)
    dst = edge_index[1].astype(np.int64)
    rel = edge_type.astype(np.int64)

    cnt = np.zeros((N, R), np.float32)
    np.add.at(cnt, (dst, rel), 1.0)
    w_edge = (1.0 / np.maximum(cnt, 1.0))[dst, rel].astype(np.float32)
    nt = node_type.astype(np.int64)

    core = dst // S
    dloc = dst - core * S
    win = dloc // WIN

    # Within-core node->position permutation balancing per-window edge
    # counts (greedy LPT by in-degree), so the cross-core max chunk count
    # per window stays at ~E_cw/128 instead of paying the variance.
    deg = cnt.sum(axis=1).astype(np.int64)            # in-degree per node
    pos_of = np.empty(N, np.int64)                    # node -> core position
    core_of = np.empty(N, np.int64)                   # node -> core
    node_at = np.empty(N, np.int64)                   # (c, pos) -> node
    caps0 = [min(WIN, S - w * WIN) for w in range(NWIN)]
    NB = NCORES * NWIN
    caps = np.tile(np.array(caps0, np.int64), NCORES)
    sums = np.zeros(NB, np.float64)
    members = [[] for _ in range(NB)]
    order_n = np.argsort(-deg, kind="stable")
    for nid in order_n:
        bsel = np.argmin(np.where(caps > 0, sums, np.inf))
        members[bsel].append(nid)
        sums[bsel] += deg[nid]
        caps[bsel] -= 1
    for c in range(NCORES):
        p0 = 0
        for w in range(NWIN):
            mem = np.array(members[c * NWIN + w], dtype=np.int64)
            node_at[c * S + p0:c * S + p0 + len(mem)] = mem
            pos_of[mem] = p0 + np.arange(len(mem))
            core_of[mem] = c
            p0 += len(mem)

    core = core_of[dst]                               # re-assigned dst core
    dloc = pos_of[dst]                                # positional dst
    win = dloc // WIN

    lut = _xg_lut()
    stA = [[None] * NWIN for _ in range(NCORES)]   # L1 (edges + self)
    stB = [[None] * NWIN for _ in range(NCORES)]   # L2/3 (edges only)
    for c in range(NCORES):
        m = core == c
        s_c, d_c, r_c, w_c, wi_c = src[m], dloc[m], rel[m], w_edge[m], win[m]
        order = np.lexsort((d_c, r_c, wi_c))
        s_c, d_c, r_c, w_c, wi_c = (a[order] for a in (s_c, d_c, r_c, w_c, wi_c))
        bounds = np.searchsorted(wi_c, np.arange(NWIN + 1))
        for w in range(NWIN):
            lo, hi = bounds[w], bounds[w + 1]
            dl = d_c[lo:hi] - w * WIN
            nwn = min(WIN, S - w * WIN)
            gids = node_at[c * S + w * WIN:c * S + w * WIN + nwn]
            stA[c][w] = (
                np.concatenate([nt[s_c[lo:hi]], nt[gids]]),
                np.concatenate([dl, np.arange(nwn)]).astype(np.float32),
                np.concatenate([w_c[lo:hi], np.ones(nwn, np.float32)]),
                np.concatenate([r_c[lo:hi], np.full(nwn, R)]),
            )
            # source row in the exchanged-x layout = positional
            sp = pos_of[s_c[lo:hi]]
            sc = core_of[s_c[lo:hi]]
            stB[c][w] = (lut[sc * S + sp], dl.astype(np.float32), w_c[lo:hi],
                         r_c[lo:hi])
    return stA, stB, cnt, node_at


def _grid(streams):
    """Shared chunk/segment structure (union over cores)."""
    nch = np.zeros(NWIN, np.int64)
    for w in range(NWIN):
        mx = max(len(streams[c][w][0]) for c in range(NCORES))
        nch[w] = max(1, math.ceil(mx / CHUNK))
    segs = [[] for _ in range(NWIN)]
    for w in range(NWIN):
        present = set()
        for c in range(NCORES):
            r_arr = streams[c][w][3]
            for k in range(int(nch[w])):
                rr = np.unique(r_arr[k * CHUNK:(k + 1) * CHUNK])
                for r in rr:
                    present.add((k, int(r)))
        segs[w] = sorted(present)
    chunk_cols = []
    seg_cols = []
    for w in range(NWIN):
        for k in range(int(nch[w])):
            chunk_cols.append((w, k))
        for (k, r) in segs[w]:
            seg_cols.append((w, k, r))
    return nch, segs, chunk_cols, seg_cols


def _fill(streams, nch, chunk_cols, seg_cols):
    CC, CS = len(chunk_cols), len(seg_cols)
    srci = np.zeros((NCORES, 128, CC), np.int32)
    dstf = np.zeros((NCORES, 128, CS), np.float32)
    wv = np.zeros((NCORES, 128, CS), np.float32)
    for c in range(NCORES):
        for j, (w, k) in enumerate(chunk_cols):
            s_arr = streams[c][w][0]
            seg = s_arr[k * CHUNK:(k + 1) * CHUNK]
            srci[c, :len(seg), j] = seg
        for j, (w, k, r) in enumerate(seg_cols):
            s_arr, d_arr, w_arr, r_arr = streams[c][w]
            sl = slice(k * CHUNK, (k + 1) * CHUNK)
            d_s, w_s, r_s = d_arr[sl], w_arr[sl], r_arr[sl]
            m = r_s == r
            kk = len(d_s)
            dstf[c, :kk, j] = np.where(m, d_s, 0.0)
            wv[c, :kk, j] = np.where(m, w_s, 0.0)
    return srci, dstf, wv


def _preprocess(node_type, edge_index, edge_type, batch):
    stA, stB, _, node_at = _streams(node_type, edge_index, edge_type, batch)
    gA = _grid(stA)
    gB = _grid(stB)
    mA = _fill(stA, gA[0], gA[2], gA[3])
    mB = _fill(stB, gB[0], gB[2], gB[3])

    bcnt = np.zeros(B, np.float64)
    np.add.at(bcnt, batch.astype(np.int64), 1.0)
    inv_b = (1.0 / np.maximum(bcnt, 1.0)).astype(np.float32)
    batchf = np.full((NCORES, 128, NHALF), -1.0, np.float32)
    invcb = np.zeros((NCORES, 128, NHALF), np.float32)
    for c in range(NCORES):
        ids = batch[node_at[c * S:(c + 1) * S]].astype(np.int64)
        for j in range(NHALF):
            seg = ids[j * 128:(j + 1) * 128]
            k = len(seg)
            batchf[c, :k, j] = seg.astype(np.float32)
            invcb[c, :k, j] = inv_b[seg]
    return gA, gB, mA, mB, batchf, invcb


def _build_program(gA, gB, CCA, CSA, CCB, CSB):
    import concourse.bass as bass
    import concourse.bacc as bacc
    import concourse.mybir as mybir
    import concourse.tile as tile
    from concourse.masks import make_identity

    f32 = mybir.dt.float32
    bf16 = mybir.dt.bfloat16
    fp8 = mybir.dt.float8e4
    i32 = mybir.dt.int32
    AF = mybir.ActivationFunctionType
    OP = mybir.AluOpType

    XT_COLS = NWIN * WIN       # 12544 (padded S)

    nc = bacc.Bacc("TRN2", target_bir_lowering=False, debug=False,
                   num_devices=NCORES)

    t_emb = nc.dram_tensor("node_emb", [V, D], bf16, kind="ExternalInput")
    t_wpack = nc.dram_tensor("wpack", [L, 128, 10 * 128], bf16,
                             kind="ExternalInput")
    t_srcA = nc.dram_tensor("srcA", [128, CCA], i32, kind="ExternalInput")
    t_dstfA = nc.dram_tensor("dstfA", [128, CSA], f32, kind="ExternalInput")
    t_wvA = nc.dram_tensor("wvA", [128, CSA], f32, kind="ExternalInput")
    t_srcB = nc.dram_tensor("srcB", [128, CCB], i32, kind="ExternalInput")
    t_dstfB = nc.dram_tensor("dstfB", [128, CSB], f32, kind="ExternalInput")
    t_wvB = nc.dram_tensor("wvB", [128, CSB], f32, kind="ExternalInput")
    t_batchf = nc.dram_tensor("batchf", [128, NHALF], f32, kind="ExternalInput")
    t_invcb = nc.dram_tensor("invcb", [128, NHALF], f32, kind="ExternalInput")
    t_iota = nc.dram_tensor("iota", [128, WIN], bf16, kind="ExternalInput")
    t_e0 = nc.dram_tensor("e0", [128, WIN], bf16, kind="ExternalInput")
    t_rw1 = nc.dram_tensor("rw1", [128, 128], f32, kind="ExternalInput")
    t_sw1 = nc.dram_tensor("sw1", [128, 128], f32, kind="ExternalInput")
    t_w2p = nc.dram_tensor("w2p", [128, 2], f32, kind="ExternalInput")
    t_b1p = nc.dram_tensor("b1p", [128, 2], f32, kind="ExternalInput")
    t_b2p = nc.dram_tensor("b2p", [64, 2], f32, kind="ExternalInput")
    t_out = nc.dram_tensor("out", [64, 2], f32, kind="ExternalOutput")

    with tile.TileContext(nc) as tc:
        with tc.tile_pool(name="static", bufs=1) as st, \
             tc.tile_pool(name="wt", bufs=2) as wtp, \
             tc.tile_pool(name="msgs", bufs=14) as msgsp, \
             tc.tile_pool(name="sel", bufs=8) as selp, \
             tc.tile_pool(name="aggsb", bufs=4) as aggsbp, \
             tc.tile_pool(name="xo", bufs=4) as xop, \
             tc.tile_pool(name="pagg", bufs=3, space="PSUM") as paggp, \
             tc.tile_pool(name="pout", bufs=2, space="PSUM") as poutp, \
             tc.tile_pool(name="ptr", bufs=2, space="PSUM") as ptrp, \
             tc.tile_pool(name="pg", bufs=1, space="PSUM") as pgp, \
             tc.tile_pool(name="dram", bufs=1, space="DRAM") as dram:

            srcA_t = st.tile([128, CCA], i32)
            dstfA_t = st.tile([128, CSA], f32)
            wvA_t = st.tile([128, CSA], f32)
            srcB_t = st.tile([128, CCB], i32)
            dstfB_t = st.tile([128, CSB], f32)
            wvB_t = st.tile([128, CSB], f32)
            batchf_t = st.tile([128, NHALF], f32)
            invcb_t = st.tile([128, NHALF], f32)
            iota_t = st.tile([128, WIN], bf16)
            e0_t = st.tile([128, WIN], bf16)
            ident_t = st.tile([128, 128], bf16)
            for dst_t, src_t in ((srcA_t, t_srcA), (dstfA_t, t_dstfA),
                                 (wvA_t, t_wvA), (srcB_t, t_srcB),
                                 (dstfB_t, t_dstfB), (wvB_t, t_wvB),
                                 (batchf_t, t_batchf), (invcb_t, t_invcb),
                                 (iota_t, t_iota), (e0_t, t_e0)):
                nc.sync.dma_start(dst_t[:], src_t[:])
            make_identity(nc, ident_t[:])

            # persistent x^T of previous / current layer (bf16)
            xT = [st.tile([128, XT_COLS], bf16, name=f"xT{i}") for i in range(2)]

            ag_in = [dram.tile([S, D], fp8, tag=f"agin{l}", name=f"agin{l}")
                     for l in range(2)]
            # non-Shared output: pieces AllGather directly into disjoint
            # slices (Shared space enforces a single writing instruction)
            ag_out = [dram.tile([N, D], fp8, tag=f"agout{l}",
                                name=f"agout{l}") for l in range(2)]
            pg = pgp.tile([128, B], f32)
            # fire piece p's AllGather a few half-windows after its rows are
            # produced so the issuing sequencer never stalls on the DMA sems;
            # pieces whose (delayed) fire point falls past the loop flush at
            # the layer end.
            DELAY_HW = 4
            fire_at = {}
            for p in range(NPIECE):
                t = PIECE_HW[p + 1] - 1 + DELAY_HW
                if t < NHALF - 1:
                    fire_at[t] = p

            def _ag_piece(l, p):
                a = 128 * PIECE_HW[p]
                nc.gpsimd.collective_compute(
                    "AllGather", mybir.AluOpType.bypass,
                    replica_groups=[list(range(NCORES))],
                    ins=[ag_in[l][a:a + PIECE_ROWS[p], :]],
                    outs=[ag_out[l][PIECE_BASE[p]:
                                    PIECE_BASE[p] + 8 * PIECE_ROWS[p], :]])

            for l in range(L):
                wtile = wtp.tile([128, 10 * 128], bf16)
                nc.sync.dma_start(wtile[:], t_wpack[l])

                if l == 0:
                    nchs, segss, chunk_cols, seg_cols = gA
                    src_t, dstf_t, wv_t = srcA_t, dstfA_t, wvA_t
                    xsrc = t_emb
                else:
                    nchs, segss, chunk_cols, seg_cols = gB
                    src_t, dstf_t, wv_t = srcB_t, dstfB_t, wvB_t
                    xsrc = ag_out[l - 1]
                ch_col = {wk: j for j, wk in enumerate(chunk_cols)}
                sg_col = {wkr: j for j, wkr in enumerate(seg_cols)}
                xT_cur = xT[l % 2]
                xT_prev = xT[(l + 1) % 2]

                for w in range(NWIN):
                    poutT = poutp.tile([128, WIN], f32)
                    nc.tensor.matmul(
                        poutT[:], lhsT=wtile[:, 9 * 128:10 * 128],
                        rhs=e0_t[:], start=True, stop=False)
                    if l > 0:
                        # root transform from SBUF-resident prev x^T
                        nc.tensor.matmul(
                            poutT[:], lhsT=wtile[:, R * 128:(R + 1) * 128],
                            rhs=xT_prev[:, w * WIN:(w + 1) * WIN],
                            start=False, stop=False)

                    # gather all chunks of this window
                    msgs_tiles = []
                    for k in range(int(nchs[w])):
                        msgs = msgsp.tile([128, 128], bf16 if l == 0 else fp8,
                                          name=f"msgs{l}_{w}_{k}", tag="msgs")
                        nc.gpsimd.indirect_dma_start(
                            out=msgs[:], out_offset=None, in_=xsrc[:],
                            in_offset=bass.IndirectOffsetOnAxis(
                                ap=src_t[:, ch_col[(w, k)]:ch_col[(w, k)] + 1],
                                axis=0))
                        msgs_tiles.append(msgs)

                    by_rel = {}
                    for (k, r) in segss[w]:
                        by_rel.setdefault(r, []).append(k)
                    rlist = sorted(by_rel.keys())
                    nrel = len(rlist)
                    for ri, r in enumerate(rlist):
                        ks = by_rel[r]
                        paggT = paggp.tile([128, WIN], f32, tag="paggT",
                                           name=f"paggT{l}_{w}_{r}")
                        for i, k in enumerate(ks):
                            j = sg_col[(w, k, r)]
                            sel = selp.tile([128, WIN], bf16,
                                            name=f"sel{l}_{w}_{r}_{i}",
                                            tag="sel")
                            nc.vector.tensor_scalar(
                                out=sel[:], in0=iota_t[:],
                                scalar1=dstf_t[:, j:j + 1],
                                scalar2=wv_t[:, j:j + 1],
                                op0=OP.is_equal, op1=OP.mult)
                            nc.tensor.matmul(
                                paggT[:], lhsT=msgs_tiles[k][:], rhs=sel[:],
                                start=(i == 0), stop=(i == len(ks) - 1))
                        aggsb = aggsbp.tile([128, WIN], bf16, tag="aggsb",
                                            name=f"aggsb{l}_{w}_{r}")
                        if r % 2 == 0:
                            nc.scalar.activation(aggsb[:], paggT[:], AF.Copy)
                        else:
                            nc.vector.tensor_copy(aggsb[:], paggT[:])
                        nc.tensor.matmul(
                            poutT[:], lhsT=wtile[:, r * 128:(r + 1) * 128],
                            rhs=aggsb[:], start=False,
                            stop=(ri == nrel - 1))

                    # relu -> persistent x^T slice (bf16)
                    nc.scalar.activation(
                        xT_cur[:, w * WIN:(w + 1) * WIN], poutT[:], AF.Relu)

                    nh = min(2, math.ceil((S - w * WIN) / 128))
                    for h in range(nh):
                        rows = min(128, S - (w * WIN + h * 128))
                        ptr = ptrp.tile([128, 128], bf16, tag="ptr",
                                        name=f"ptr{l}_{w}_{h}")
                        nc.tensor.transpose(
                            ptr[:],
                            xT_cur[:, w * WIN + h * 128:w * WIN + (h + 1) * 128],
                            ident_t[:])
                        xo = xop.tile([128, 128], fp8 if l < 2 else bf16,
                                      tag="xo", name=f"xo{l}_{w}_{h}")
                        nc.vector.tensor_copy(xo[:], ptr[:])
                        if l < 2:
                            hw_ = w * 2 + h
                            nc.sync.dma_start(
                                ag_in[l][w * WIN + h * 128:
                                         w * WIN + h * 128 + rows, :],
                                xo[:rows, :])
                            if hw_ in fire_at:
                                _ag_piece(l, fire_at[hw_])
                        else:
                            hw_ = w * 2 + h
                            selb = selp.tile([128, B], bf16, tag="selb",
                                             name=f"selb{w}_{h}")
                            nc.vector.tensor_scalar(
                                out=selb[:], in0=iota_t[:, :B],
                                scalar1=batchf_t[:, hw_:hw_ + 1],
                                scalar2=invcb_t[:, hw_:hw_ + 1],
                                op0=OP.is_equal, op1=OP.mult)
                            nc.tensor.matmul(
                                pg[:], lhsT=xo[:], rhs=selb[:],
                                start=(hw_ == 0), stop=(hw_ == NHALF - 1))

                if l < 2:
                    for p in range(NPIECE):
                        if PIECE_HW[p + 1] - 1 + DELAY_HW >= NHALF - 1:
                            _ag_piece(l, p)

            # heads
            rw1_t = st.tile([128, 128], f32)
            sw1_t = st.tile([128, 128], f32)
            w2p_t = st.tile([128, 2], f32)
            b1p_t = st.tile([128, 2], f32)
            b2p_t = st.tile([64, 2], f32)
            nc.sync.dma_start(rw1_t[:], t_rw1[:])
            nc.sync.dma_start(sw1_t[:], t_sw1[:])
            nc.sync.dma_start(w2p_t[:], t_w2p[:])
            nc.sync.dma_start(b1p_t[:], t_b1p[:])
            nc.sync.dma_start(b2p_t[:], t_b2p[:])

            pgsb = st.tile([128, B], f32)
            nc.vector.tensor_copy(pgsb[:], pg[:])
            ar_in = dram.tile([128, B], f32, tag="arin")
            ar_out = dram.tile([128, B], f32, addr_space="Shared", tag="arout")
            nc.sync.dma_start(ar_in[:], pgsb[:])
            nc.gpsimd.collective_compute(
                "AllReduce", mybir.AluOpType.add,
                replica_groups=[list(range(NCORES))],
                ins=[ar_in[:]], outs=[ar_out[:]])
            gT = st.tile([128, B], f32)
            nc.sync.dma_start(gT[:], ar_out[:])

            ph2 = ptrp.tile([64, 2], f32, tag="ptr")
            for ci, w1t in enumerate((rw1_t, sw1_t)):
                ph = paggp.tile([128, B], f32, tag="paggT",
                                name=f"ph{ci}")
                nc.tensor.matmul(ph[:], lhsT=w1t[:], rhs=gT[:],
                                 start=True, stop=True)
                hT = st.tile([128, B], f32, tag=f"hT{ci}", name=f"hT{ci}")
                nc.scalar.activation(hT[:], ph[:], AF.Relu,
                                     bias=b1p_t[:, ci:ci + 1])
                nc.tensor.matmul(ph2[:, ci:ci + 1], lhsT=hT[:],
                                 rhs=w2p_t[:, ci:ci + 1],
                                 start=True, stop=True)
            outsb = st.tile([64, 2], f32)
            nc.vector.tensor_add(outsb[:], ph2[:], b2p_t[:])
            nc.sync.dma_start(t_out[:], outsb[:])

    nc.compile()
    return nc


def kernel(node_type, edge_index, edge_type, batch, node_emb, rel_w, root_w,
           bias, risk_w1, risk_b1, risk_w2, risk_b2, safe_w1, safe_b1,
           safe_w2, safe_b2):
    global LAST_RESULTS
    import concourse.bass_utils as bass_utils

    node_type = np.asarray(node_type, np.int32)
    edge_index = np.asarray(edge_index, np.int32)
    edge_type = np.asarray(edge_type, np.int32)
    batch = np.asarray(batch, np.int32)
    node_emb = np.asarray(node_emb, np.float32)
    rel_w = np.asarray(rel_w, np.float32)
    root_w = np.asarray(root_w, np.float32)
    bias_np = np.asarray(bias, np.float32)

    gA, gB, mA, mB, batchf, invcb = _preprocess(
        node_type, edge_index, edge_type, batch)
    srcA, dstfA, wvA = mA
    srcB, dstfB, wvB = mB

    nc = _build_program(gA, gB, srcA.shape[2], dstfA.shape[2],
                        srcB.shape[2], dstfB.shape[2])

    bf = ml_dtypes.bfloat16
    wpack = np.zeros((L, 10, 128, 128), np.float32)
    wpack[:, :R] = rel_w
    wpack[:, R] = root_w
    wpack[:, 9, 0, :] = bias_np
    wpack = np.ascontiguousarray(wpack.transpose(0, 2, 1, 3)).reshape(
        L, 128, 10 * 128).astype(bf)

    iota = np.tile(np.arange(WIN, dtype=np.float32), (128, 1)).astype(bf)
    e0 = np.zeros((128, WIN), np.float32)
    e0[0, :] = 1.0
    e0 = e0.astype(bf)
    w2p = np.stack([np.asarray(risk_w2, np.float32)[:, 0],
                    np.asarray(safe_w2, np.float32)[:, 0]], axis=1)
    b1p = np.stack([np.asarray(risk_b1, np.float32),
                    np.asarray(safe_b1, np.float32)], axis=1)
    b2p = np.stack([np.full(64, np.float32(np.asarray(risk_b2)[0])),
                    np.full(64, np.float32(np.asarray(safe_b2)[0]))], axis=1)

    shared = dict(node_emb=node_emb.astype(bf), wpack=wpack, iota=iota, e0=e0,
                  rw1=np.asarray(risk_w1, np.float32),
                  sw1=np.asarray(safe_w1, np.float32),
                  w2p=w2p, b1p=b1p, b2p=b2p)
    in_maps = []
    for c in range(NCORES):
        m = dict(shared)
        m.update(srcA=srcA[c], dstfA=dstfA[c], wvA=wvA[c],
                 srcB=srcB[c], dstfB=dstfB[c], wvB=wvB[c],
                 batchf=batchf[c], invcb=invcb[c])
        in_maps.append(m)

    trace = os.environ.get("KERNEL_TRACE", "0") == "1"
    res = bass_utils.run_bass_kernel_spmd(
        nc, in_maps, core_ids=list(range(NCORES)), trace=trace)
    LAST_RESULTS = res
    out = res.results[0]["out"]
    return out[:, 0].copy(), out[:, 1].copy()
